# revision 1
# baseline (speedup 1.0000x reference)
"""Trainium2 Bass kernel for nn_Decoder (3-stage point-cloud decoder:
gather-upsample + concat-skip + 1x1conv (+BN+LeakyReLU) x2, final plain conv).

Strategy (8-core SPMD):
  - Gather commutes with the per-row linear map: feats[idx] @ Wa == (feats@Wa)[idx].
    Stages 1-2 pre-multiply the gathered operand into small z-tables and
    row-gather them with gpsimd.dma_gather (256B rows, int16 local indices).
  - Points of each stage are sharded by the core that owns their gather source,
    so every gather is core-local. No AllGathers; only two tiny BN-stats
    AllReduces.
  - Stage 3 has no gather at all: stage-2 points are host-grouped by their
    stage-3 fan-out class, so the upsample becomes a handful of static
    stride-0 broadcast-adds (z3T columns repeated k times each) with
    compile-time class tables. Stage-3 runs channel-major; the host
    transposes the [34, n] output back.
  - Skips are host-transposed to channel-major and used as matmul stationaries
    (stages 1-2, point-major psum) or moving operands (stage 3, weights
    stationary).
  - BN stats in channel-major (free-axis reduce / ACT square-accumulate);
    BN-affine+LeakyReLU is a single ACT op with per-partition scale/bias.
    b1/b2 cancel inside BN exactly; b3 is folded into the z3 table.
  - BN pad rows are exactly zero (zero z-table rows + zero skip columns), so
    global stats divide by the true N.
"""

import sys

sys.path.insert(0, "/opt/trn_rl_repo")

import numpy as np

from concourse import bacc, bass, bass_utils, masks, mybir, tile

dt = mybir.dt
AF = mybir.ActivationFunctionType
OP = mybir.AluOpType
AX = mybir.AxisListType

NCORES = 8
EPS = 1e-5
SLOPE = 0.01

N3, N2, N1, N0 = 4096, 16384, 65536, 262144
C3 = 512  # coarse bucket size = N3 // NCORES

GMAX = 1024  # max indices per dma_gather (SWDGE ring holds ~64 data descs)
S1CMAX = 10240  # max stage-3 skip chunk columns resident in SBUF


def _ceil_to(x, m):
    return ((x + m - 1) // m) * m


def _gplan(p0, p1):
    out = []
    off = p0
    while off < p1:
        ln = min(GMAX, p1 - off)
        out.append((off, ln))
        off += ln
    return out


def _cls_layout(M):
    """M: tuple of per-fanout-class source-slot counts (index = fanout k).
    Returns (src_off[k], out_off[k], n2p, n3p, s1chunks) where s1chunks is a
    list of (out0, olen, [(k, out_piece0, src0, ck), ...]) groups of whole
    class blocks limited to S1CMAX output columns, pieces of <=512 cols."""
    K = len(M)
    src_off = [0] * K
    out_off = [0] * K
    o = 0
    for k in range(K):
        src_off[k] = o
        o += M[k]
    n2p = o
    o = 0
    for k in range(1, K):
        out_off[k] = o
        o += M[k] * k
    n3p = o

    chunks = []
    cur0, curlen, curpieces = 0, 0, []
    for k in range(1, K):
        blk = M[k] * k
        if blk == 0:
            continue
        assert blk <= S1CMAX, f"class {k} block {blk} exceeds S1CMAX"
        if curlen + blk > S1CMAX and curlen > 0:
            chunks.append((cur0, curlen, curpieces))
            cur0, curlen, curpieces = out_off[k], 0, []
        ck_max = (512 // k) * k
        done = 0
        while done < blk:
            ck = min(ck_max, blk - done)
            curpieces.append(
                (k, out_off[k] + done, src_off[k] + done // k, ck)
            )
            done += ck
        curlen += blk
    if curlen > 0:
        chunks.append((cur0, curlen, curpieces))
    return src_off, out_off, n2p, n3p, chunks


def _wrap_idx(idx, plan):
    """[n] int -> [128, n//16] int16, wrapped per plan block, replicated
    across 16-partition groups (dma_gather idx layout)."""
    n = len(idx)
    out = np.empty((128, n // 16), np.int16)
    for off, ln in plan:
        w = idx[off : off + ln].reshape(ln // 16, 16).T.astype(np.int16)
        out[:, off // 16 : (off + ln) // 16] = np.tile(w, (8, 1))
    return out


def _bucket(owner, ncores):
    """owner: [n] core id per element -> (perm lists, positions, counts)."""
    order = np.argsort(owner, kind="stable")
    counts = np.bincount(owner, minlength=ncores)
    splits = np.split(order, np.cumsum(counts)[:-1])
    pos = np.empty(len(owner), np.int64)
    for c in range(ncores):
        pos[splits[c]] = np.arange(counts[c])
    return splits, pos, counts


# ---------------------------------------------------------------------------
# device program
# ---------------------------------------------------------------------------

PHASES = ["z1", "g1", "y1", "t1", "sm1", "ar1", "st1", "z2", "st2", "z3", "st3"]


def _build_program(n1p, Mtuple, nch2, stop_after=None):
    lim = PHASES.index(stop_after) if stop_after else len(PHASES) - 1

    def on(ph):
        return PHASES.index(ph) <= lim

    src_off, out_off, n2p, n3p, s1chunks = _cls_layout(Mtuple)

    nc = bacc.Bacc(
        "TRN2",
        target_bir_lowering=False,
        debug=False,
        num_devices=NCORES,
        num_swdge_queues=4,
    )

    f32 = dt.float32
    i16 = dt.int16

    T1 = n1p // 128
    T2 = n2p // 128
    C2CH = n2p // nch2  # stage-2 skip/gather chunk (points)

    # ---- I/O ----
    featsT_h = nc.dram_tensor("featsT", [258, C3], f32, kind="ExternalInput")
    s3T_h = nc.dram_tensor("s3T", [512, n1p], f32, kind="ExternalInput")
    s2T_h = nc.dram_tensor("s2T", [256, n2p], f32, kind="ExternalInput")
    s1T_h = nc.dram_tensor("s1T", [128, n3p], f32, kind="ExternalInput")
    gi1_h = nc.dram_tensor("gi1", [128, n1p // 16], i16, kind="ExternalInput")
    gi2_h = nc.dram_tensor("gi2", [128, n2p // 16], i16, kind="ExternalInput")
    W1a_h = nc.dram_tensor("W1a", [258, 192], f32, kind="ExternalInput")
    W1b_h = nc.dram_tensor("W1b", [512, 129], f32, kind="ExternalInput")
    W2a_h = nc.dram_tensor("W2a", [129, 64], f32, kind="ExternalInput")
    W2b_h = nc.dram_tensor("W2b", [256, 64], f32, kind="ExternalInput")
    W3a_h = nc.dram_tensor("W3a", [64, 64], f32, kind="ExternalInput")
    W3b_h = nc.dram_tensor("W3b", [128, 34], f32, kind="ExternalInput")
    bn1_h = nc.dram_tensor("bn1", [129, 2], f32, kind="ExternalInput")  # g, be
    bn2_h = nc.dram_tensor("bn2", [64, 2], f32, kind="ExternalInput")
    b3_h = nc.dram_tensor("b3", [64, 1], f32, kind="ExternalInput")
    out_h = nc.dram_tensor("out", [34, n3p], f32, kind="ExternalOutput")

    def bn_scalars(sb, nc, stats, gbe, n_true, P, name):
        """stats [P,2]=(sum,sumsq) -> s,t tiles [P,1]: s=g*rsqrt(var+eps),
        t=be-mean*s."""
        mean = sb.tile([P, 1], f32, tag=f"{name}_mean")
        ms = sb.tile([P, 1], f32, tag=f"{name}_ms")
        nc.vector.tensor_scalar(mean[:], stats[:, 0:1], 1.0 / n_true, None, OP.mult)
        nc.vector.tensor_scalar(ms[:], stats[:, 1:2], 1.0 / n_true, None, OP.mult)
        var = sb.tile([P, 1], f32, tag=f"{name}_var")
        nc.vector.tensor_tensor(var[:], mean[:], mean[:], OP.mult)
        nc.vector.tensor_tensor(var[:], ms[:], var[:], OP.subtract)
        nc.vector.tensor_scalar(var[:], var[:], EPS, None, OP.add)
        std = sb.tile([P, 1], f32, tag=f"{name}_std")
        nc.scalar.activation(std[:], var[:], AF.Sqrt)
        s = sb.tile([P, 1], f32, tag=f"{name}_s")
        nc.vector.reciprocal(s[:], std[:])
        nc.vector.tensor_tensor(s[:], s[:], gbe[:, 0:1], OP.mult)
        t = sb.tile([P, 1], f32, tag=f"{name}_t")
        nc.vector.tensor_tensor(t[:], mean[:], s[:], OP.mult)
        nc.vector.tensor_tensor(t[:], gbe[:, 1:2], t[:], OP.subtract)
        return s, t

    def stats_of(sb, nc, x, P, n, name):
        """x [P, n] -> stats [P, 2] = (sum, sumsq). Square pass chunked so the
        scratch stays small."""
        stats = sb.tile([P, 2], f32, tag=f"{name}_stats")
        nc.vector.tensor_reduce(stats[:, 0:1], x[:], AX.X, OP.add)
        CH = 512
        nchunk = (n + CH - 1) // CH
        ssq = sb.tile([P, nchunk], f32, tag=f"{name}_ssqp")
        for k in range(nchunk):
            c0, c1 = k * CH, min((k + 1) * CH, n)
            scr = sb.tile([P, CH], f32, tag=f"{name}_sqscr")
            nc.scalar.activation(
                scr[:, : c1 - c0], x[:, c0:c1], AF.Square, accum_out=ssq[:, k : k + 1]
            )
        nc.vector.tensor_reduce(stats[:, 1:2], ssq[:], AX.X, OP.add)
        return stats

    with tile.TileContext(nc) as tc:
        from contextlib import ExitStack

        octx = ExitStack()
        with octx:
            sb = octx.enter_context(tc.tile_pool(name="persist", bufs=1))
            dram = octx.enter_context(tc.tile_pool(name="dram", bufs=1, space="DRAM"))

            ident = sb.tile([128, 128], f32)
            masks.make_identity(nc, ident[:])
            zrow = sb.tile([1, 192], f32)
            nc.gpsimd.memset(zrow[:], 0.0)

            # weights / params
            W1a = sb.tile([128, 2, 192], f32)
            W1ax = sb.tile([2, 192], f32)
            nc.sync.dma_start(W1a[:, 0, :], W1a_h.ap()[0:128, :])
            nc.sync.dma_start(W1a[:, 1, :], W1a_h.ap()[128:256, :])
            nc.sync.dma_start(W1ax[:], W1a_h.ap()[256:258, :])
            W1b = sb.tile([128, 4, 129], f32)
            for k in range(4):
                nc.sync.dma_start(W1b[:, k, :], W1b_h.ap()[k * 128 : (k + 1) * 128, :])
            W2a = sb.tile([128, 64], f32)
            W2ax = sb.tile([1, 64], f32)
            nc.sync.dma_start(W2a[:], W2a_h.ap()[0:128, :])
            nc.sync.dma_start(W2ax[:], W2a_h.ap()[128:129, :])
            W2b = sb.tile([128, 2, 64], f32)
            for k in range(2):
                nc.sync.dma_start(W2b[:, k, :], W2b_h.ap()[k * 128 : (k + 1) * 128, :])
            W3a = sb.tile([64, 64], f32)
            nc.sync.dma_start(W3a[:], W3a_h.ap())
            W3b = sb.tile([128, 34], f32)
            nc.sync.dma_start(W3b[:], W3b_h.ap())
            bn1 = sb.tile([128, 2], f32)
            bn1x = sb.tile([1, 2], f32)
            nc.sync.dma_start(bn1[:], bn1_h.ap()[0:128, :])
            nc.sync.dma_start(bn1x[:], bn1_h.ap()[128:129, :])
            bn2 = sb.tile([64, 2], f32)
            nc.sync.dma_start(bn2[:], bn2_h.ap())
            b3p = sb.tile([64, 1], f32)
            nc.sync.dma_start(b3p[:], b3_h.ap())

            # gather index tiles
            gi1 = sb.tile([128, n1p // 16], i16)
            nc.sync.dma_start(gi1[:], gi1_h.ap())
            gi2 = sb.tile([128, n2p // 16], i16)
            nc.sync.dma_start(gi2[:], gi2_h.ap())

            featsT = sb.tile([128, 2, C3], f32)
            featsTx = sb.tile([2, C3], f32)
            nc.sync.dma_start(featsT[:, 0, :], featsT_h.ap()[0:128, :])
            nc.sync.dma_start(featsT[:, 1, :], featsT_h.ap()[128:256, :])
            nc.sync.dma_start(featsTx[:], featsT_h.ap()[256:258, :])

            # z tables (stages 1-2 in DRAM for dma_gather; z3T in SBUF)
            z1d = dram.tile([513, 192], f32)
            z2d = dram.tile([n1p + 1, 64], f32)

            # cross-stage SBUF. z3T overwrites x2T in place chunk-by-chunk
            # (each x2T column is read once by the z3T matmul, then dead).
            x2T = sb.tile([64, n2p], f32)
            z3T = x2T

            # ---------------- phase B: z1 = featsT.T @ W1a ----------------
            if on("z1"):
                with (
                    nc.named_scope("ph_z1"),
                    tc.tile_pool(name="z1ps", bufs=2, space="PSUM") as z1ps,
                    tc.tile_pool(name="z1sb", bufs=2) as z1sb,
                ):
                    for t in range(4):
                        ps = z1ps.tile([128, 192], f32, tag="ps")
                        c0 = t * 128
                        nc.tensor.matmul(
                            ps[:], featsT[:, 0, c0 : c0 + 128], W1a[:, 0, :],
                            start=True, stop=False,
                        )
                        nc.tensor.matmul(
                            ps[:], featsT[:, 1, c0 : c0 + 128], W1a[:, 1, :],
                            start=False, stop=False,
                        )
                        nc.tensor.matmul(
                            ps[:], featsTx[:, c0 : c0 + 128], W1ax[:],
                            start=False, stop=True,
                        )
                        zt = z1sb.tile([128, 192], f32, tag="zt")
                        nc.vector.tensor_copy(zt[:], ps[:])
                        nc.sync.dma_start(z1d[c0 : c0 + 128, :], zt[:])
                    nc.sync.dma_start(z1d[512:513, :], zrow[:])

            # ---------------- stage 1 + z2 (x1T scope) ----------------
            x1ctx = ExitStack()
            x1pool = x1ctx.enter_context(tc.tile_pool(name="x1pool", bufs=1))
            x1T = x1pool.tile([128, n1p], f32)
            x1Tx = x1pool.tile([1, n1p], f32)

            if on("g1"):
                with (
                    nc.named_scope("ph_st1"),
                    tc.tile_pool(name="st1", bufs=1) as st1,
                    tc.tile_pool(name="yps", bufs=2, space="PSUM") as yps,
                    tc.tile_pool(name="tps", bufs=2, space="PSUM") as tps,
                ):
                    s3T = st1.tile([128, 4, n1p], f32)
                    for k in range(4):
                        nc.sync.dma_start(
                            s3T[:, k, :], s3T_h.ap()[k * 128 : (k + 1) * 128, :]
                        )
                    zg1 = st1.tile([128, T1, 192], f32)
                    for qi, (off, ln) in enumerate(_gplan(0, n1p)):
                        nc.gpsimd.dma_gather(
                            zg1[:, off // 128 : (off + ln) // 128, :],
                            z1d[:],
                            gi1[:, off // 16 : (off + ln) // 16],
                            ln,
                            ln,
                            192,
                            elem_step=192,
                            queue_num=qi % 4,
                        )
                    y1 = st1.tile([128, T1, 129], f32)
                    for t in range(T1 if on("y1") else 0):
                        c0 = t * 128
                        ps = yps.tile([128, 129], f32, tag="y1ps")
                        for k in range(4):
                            nc.tensor.matmul(
                                ps[:],
                                s3T[:, k, c0 : c0 + 128],
                                W1b[:, k, :],
                                start=(k == 0),
                                stop=(k == 3),
                            )
                        nc.vector.tensor_tensor(
                            y1[:, t, :], ps[:], zg1[:, t, 0:129], OP.add
                        )
                    # transpose to channel-major
                    y1T = st1.tile([128, n1p], f32)
                    y1Tx = st1.tile([1, n1p], f32)
                    for t in range(T1 if on("t1") else 0):
                        c0 = t * 128
                        tp = tps.tile([128, 128], f32, tag="tp1")
                        nc.tensor.transpose(tp[:], y1[:, t, 0:128], ident[:])
                        nc.scalar.activation(y1T[:, c0 : c0 + 128], tp[:], AF.Copy)
                        tpx = tps.tile([1, 128], f32, tag="tp1x")
                        nc.tensor.transpose(tpx[:], y1[:, t, 128:129], ident[:])
                        nc.scalar.activation(y1Tx[:, c0 : c0 + 128], tpx[:], AF.Copy)

                    # stats + AllReduce
                    if on("sm1"):
                        st_m = stats_of(sb, nc, y1T, 128, n1p, "bn1m")
                        st_x = stats_of(sb, nc, y1Tx, 1, n1p, "bn1x")
                    if on("ar1"):
                        ar_in = dram.tile([129, 2], f32, tag="ar1i")
                        ar_out = dram.tile([129, 2], f32, tag="ar1o")
                        nc.gpsimd.dma_start(ar_in[0:128, :], st_m[:])
                        nc.gpsimd.dma_start(ar_in[128:129, :], st_x[:])
                        nc.gpsimd.collective_compute(
                            "AllReduce",
                            OP.add,
                            ins=[ar_in.opt()],
                            outs=[ar_out.opt()],
                            replica_groups=[list(range(NCORES))],
                        )
                    if on("st1"):
                        rst_m = sb.tile([128, 2], f32)
                        rst_x = sb.tile([1, 2], f32)
                        nc.sync.dma_start(rst_m[:], ar_out[0:128, :])
                        nc.sync.dma_start(rst_x[:], ar_out[128:129, :])
                        s_m, t_m = bn_scalars(sb, nc, rst_m, bn1, float(N2), 128, "bn1m")
                        s_x, t_x = bn_scalars(sb, nc, rst_x, bn1x, float(N2), 1, "bn1x")
                        nc.scalar.activation(
                            x1T[:], y1T[:], AF.Lrelu, bias=t_m[:], scale=s_m[:], alpha=SLOPE
                        )
                        nc.scalar.activation(
                            x1Tx[:], y1Tx[:], AF.Lrelu, bias=t_x[:], scale=s_x[:], alpha=SLOPE
                        )

            # ---------------- z2 = x1T.T @ W2a ----------------
            if on("z2"):
                with (
                    nc.named_scope("ph_z2"),
                    tc.tile_pool(name="z2ps", bufs=2, space="PSUM") as z2ps,
                    tc.tile_pool(name="z2sb", bufs=2) as z2sb,
                ):
                    g = 0
                    while g * 8 < T1:
                        jg = min(8, T1 - g * 8)
                        ps = z2ps.tile([128, 512], f32, tag="ps")
                        for j in range(jg):
                            c0 = (g * 8 + j) * 128
                            nc.tensor.matmul(
                                ps[:, j * 64 : (j + 1) * 64],
                                x1T[:, c0 : c0 + 128],
                                W2a[:],
                                start=True,
                                stop=False,
                            )
                            nc.tensor.matmul(
                                ps[:, j * 64 : (j + 1) * 64],
                                x1Tx[:, c0 : c0 + 128],
                                W2ax[:],
                                start=False,
                                stop=True,
                            )
                        zt = z2sb.tile([128, 512], f32, tag="zt")
                        nc.vector.tensor_copy(zt[:, : jg * 64], ps[:, : jg * 64])
                        # contiguous store: z2d row g*1024 + p*jg + j holds
                        # point g*1024 + j*128 + p (host remaps gather idxs)
                        dview = (
                            z2d[g * 1024 : g * 1024 + jg * 128, :]
                            .rearrange("(p j) c -> p (j c)", p=128, j=jg)
                        )
                        nc.sync.dma_start(dview, zt[:, : jg * 64])
                        g += 1
                    nc.sync.dma_start(z2d[n1p : n1p + 1, :], zrow[:, 0:64])
            x1ctx.close()

            # ---------------- stage 2 ----------------
            if on("st2"):
                with (
                    nc.named_scope("ph_st2"),
                    tc.tile_pool(name="st2", bufs=1) as st2,
                    tc.tile_pool(name="s2chunk", bufs=2) as s2chunk,
                    tc.tile_pool(name="yps2", bufs=4, space="PSUM") as yps2,
                    tc.tile_pool(name="tps2", bufs=4, space="PSUM") as tps2,
                ):
                    zg2 = st2.tile([128, T2, 64], f32)
                    y2 = st2.tile([128, T2, 64], f32)
                    y2T = st2.tile([64, n2p], f32)
                    for ch in range(nch2):
                        p0 = ch * C2CH
                        for off, ln in _gplan(p0, p0 + C2CH):
                            nc.gpsimd.dma_gather(
                                zg2[:, off // 128 : (off + ln) // 128, :],
                                z2d[:],
                                gi2[:, off // 16 : (off + ln) // 16],
                                ln,
                                ln,
                                64,
                                elem_step=64,
                                queue_num=(off // GMAX) % 4,
                            )
                        s2c = s2chunk.tile([128, 2, C2CH], f32, tag="s2c")
                        for k in range(2):
                            nc.sync.dma_start(
                                s2c[:, k, :],
                                s2T_h.ap()[k * 128 : (k + 1) * 128, p0 : p0 + C2CH],
                            )
                        for tl in range(C2CH // 128):
                            t = p0 // 128 + tl
                            ps = yps2.tile([128, 64], f32, tag="y2ps")
                            for k in range(2):
                                nc.tensor.matmul(
                                    ps[:],
                                    s2c[:, k, tl * 128 : (tl + 1) * 128],
                                    W2b[:, k, :],
                                    start=(k == 0),
                                    stop=(k == 1),
                                )
                            nc.vector.tensor_tensor(y2[:, t, :], ps[:], zg2[:, t, :], OP.add)
                            tp = tps2.tile([64, 128], f32, tag="tp2")
                            nc.tensor.transpose(tp[:], y2[:, t, :], ident[:])
                            nc.scalar.activation(
                                y2T[:, t * 128 : (t + 1) * 128], tp[:], AF.Copy
                            )

                    st2s = stats_of(sb, nc, y2T, 64, n2p, "bn2")
                    ar2_in = dram.tile([64, 2], f32, tag="ar2i")
                    ar2_out = dram.tile([64, 2], f32, tag="ar2o")
                    nc.gpsimd.dma_start(ar2_in[:], st2s[:])
                    nc.gpsimd.collective_compute(
                        "AllReduce",
                        OP.add,
                        ins=[ar2_in.opt()],
                        outs=[ar2_out.opt()],
                        replica_groups=[list(range(NCORES))],
                    )
                    rst2 = sb.tile([64, 2], f32)
                    nc.sync.dma_start(rst2[:], ar2_out[:])
                    s2s, t2s = bn_scalars(sb, nc, rst2, bn2, float(N1), 64, "bn2")
                    nc.scalar.activation(
                        x2T[:], y2T[:], AF.Lrelu, bias=t2s[:], scale=s2s[:], alpha=SLOPE
                    )

            # ---------------- z3T = W3a.T @ x2T + b3 (SBUF, channel-major) --
            if on("z3"):
                with (
                    nc.named_scope("ph_z3"),
                    tc.tile_pool(name="z3ps", bufs=2, space="PSUM") as z3ps,
                ):
                    for chn in range(n2p // 512):
                        sl0 = chn * 512
                        ps = z3ps.tile([64, 512], f32, tag="ps")
                        nc.tensor.matmul(
                            ps[:], W3a[:], x2T[:, sl0 : sl0 + 512],
                            start=True, stop=True,
                        )
                        nc.scalar.activation(
                            z3T[:, sl0 : sl0 + 512], ps[:], AF.Identity, bias=b3p[:]
                        )

            # ---------------- stage 3: class-expansion, channel-major -------
            if on("st3"):
                with (
                    nc.named_scope("ph_st3"),
                    tc.tile_pool(name="s1chunk", bufs=2) as s1chunk,
                    tc.tile_pool(name="outp", bufs=4) as outp,
                    tc.tile_pool(name="yps3", bufs=4, space="PSUM") as yps3,
                ):
                    for cols0, clen, pieces in s1chunks:
                        s1c = s1chunk.tile([128, S1CMAX], f32, tag="s1c")
                        nc.sync.dma_start(
                            s1c[:, :clen], s1T_h.ap()[:, cols0 : cols0 + clen]
                        )
                        for k, g0, s0, ck in pieces:
                            nj = ck // k
                            ps = yps3.tile([34, 512], f32, tag="y3ps")
                            nc.tensor.matmul(
                                ps[:, :ck],
                                W3b[:],
                                s1c[:, g0 - cols0 : g0 - cols0 + ck],
                                start=True,
                                stop=True,
                            )
                            ot = outp.tile([34, 512], f32, tag="ot")
                            nc.vector.tensor_tensor(
                                ot[:, :ck].rearrange("p (j i) -> p j i", i=k),
                                ps[:, :ck].rearrange("p (j i) -> p j i", i=k),
                                z3T[0:34, s0 : s0 + nj]
                                .unsqueeze(2)
                                .broadcast_to([34, nj, k]),
                                OP.add,
                            )
                            nc.sync.dma_start(out_h.ap()[:, g0 : g0 + ck], ot[:, :ck])

    nc.compile()
    return nc


# ---------------------------------------------------------------------------
# host wrapper
# ---------------------------------------------------------------------------

_CACHE = {}


def _get_program(key, *args):
    if key not in _CACHE:
        _CACHE[key] = _build_program(*args)
    return _CACHE[key]


def prepare(
    feats,
    skip1,
    skip2,
    skip3,
    idx1,
    idx2,
    idx3,
    W1,
    b1,
    g1,
    be1,
    W2,
    b2,
    g2,
    be2,
    W3,
    b3,
):
    """Host-side sharding/layout. Returns (build_key, in_maps, outmaps)."""
    feats = np.asarray(feats, np.float32)
    skip1 = np.asarray(skip1, np.float32)
    skip2 = np.asarray(skip2, np.float32)
    skip3 = np.asarray(skip3, np.float32)
    idx1 = np.asarray(idx1, np.int64)
    idx2 = np.asarray(idx2, np.int64)
    idx3 = np.asarray(idx3, np.int64)
    W1 = np.asarray(W1, np.float32)
    W2 = np.asarray(W2, np.float32)
    W3 = np.asarray(W3, np.float32)
    b3 = np.asarray(b3, np.float32)
    g1 = np.asarray(g1, np.float32)
    be1 = np.asarray(be1, np.float32)
    g2 = np.asarray(g2, np.float32)
    be2 = np.asarray(be2, np.float32)

    # ---- locality sharding ----
    own1 = idx1 // C3  # owner core of each stage-1 point
    P1, pos1, cnt1 = _bucket(own1, NCORES)
    own2 = own1[idx2]  # owner of each stage-2 point = owner of its source
    P2raw, _, cnt2 = _bucket(own2, NCORES)

    n1p = _ceil_to(cnt1.max(), 128)
    NCH2 = 8

    # ---- stage-3 fanout classes over stage-2 points ----
    fan = np.bincount(idx3, minlength=N1)  # global fanout per stage-2 point
    KMAX = int(fan.max())
    cnt_ck = np.zeros((NCORES, KMAX + 1), np.int64)
    for c in range(NCORES):
        cnt_ck[c] = np.bincount(fan[P2raw[c]], minlength=KMAX + 1)
    M = cnt_ck.max(axis=0)
    # pad class 0 so n2p is a multiple of 2048 (covers 512 psum chunks and
    # NCH2 gather chunks of 128-multiples)
    M[0] += _ceil_to(int(M.sum()), 2048) - int(M.sum())
    Mtuple = tuple(int(x) for x in M)

    src_off, out_off, n2p, n3p, _ = _cls_layout(Mtuple)

    # stage-2 slot assignment: class-grouped per core
    slot2 = np.full(N1, -1, np.int64)  # global stage-2 point -> core slot
    P2 = []
    for c in range(NCORES):
        pts = P2raw[c]
        order = np.argsort(fan[pts], kind="stable")
        pts_sorted = pts[order]
        # within each class, consecutive slots starting at src_off[k]
        ks = fan[pts_sorted]
        # position within class
        slots = np.empty(len(pts), np.int64)
        for k in range(KMAX + 1):
            m = ks == k
            slots[m] = src_off[k] + np.arange(int(m.sum()))
        slot2[pts_sorted] = slots
        P2.append(pts_sorted)

    # stage-3 output mapping (CSR over idx3 by source)
    order3 = np.argsort(idx3, kind="stable")
    start = np.zeros(N1 + 1, np.int64)
    np.cumsum(fan, out=start[1:])

    outmaps = []  # per core: out col -> original stage-3 point (-1 pad)
    for c in range(NCORES):
        omap = np.full(n3p, -1, np.int64)
        pts = P2[c]
        ks = fan[pts]
        for k in range(1, KMAX + 1):
            srcs = pts[ks == k]  # already in slot order
            nk = len(srcs)
            if nk == 0:
                continue
            # gatherer stage-3 points for each src, k each
            gidx = (start[srcs][:, None] + np.arange(k)[None, :]).reshape(-1)
            omap[out_off[k] : out_off[k] + nk * k] = order3[gidx]
        outmaps.append(omap)

    key = (n1p, Mtuple, NCH2)

    # shared weights
    W1a = np.zeros((258, 192), np.float32)
    W1a[:, :129] = W1[:258]
    W1b = np.ascontiguousarray(W1[258:770])
    W2a = np.ascontiguousarray(W2[:129])
    W2b = np.ascontiguousarray(W2[129:385])
    W3a = np.zeros((64, 64), np.float32)
    W3a[:, :34] = W3[:64]
    W3b = np.ascontiguousarray(W3[64:192])
    bn1 = np.stack([g1, be1], 1)
    bn2 = np.stack([g2, be2], 1)
    b3p = np.zeros((64, 1), np.float32)
    b3p[:34, 0] = b3

    featsTf = np.ascontiguousarray(feats.T)  # [258, 4096]
    s3Tf = skip3.T
    s2Tf = skip2.T
    s1Tf = skip1.T

    pl1 = _gplan(0, n1p)
    pl2 = []
    for c2 in range(NCH2):
        pl2 += _gplan(c2 * (n2p // NCH2), (c2 + 1) * (n2p // NCH2))

    in_maps = []
    for c in range(NCORES):
        p1, p2 = P1[c], P2[c]
        k1, k2 = len(p1), len(p2)
        omap = outmaps[c]

        s3T = np.zeros((512, n1p), np.float32)
        s3T[:, :k1] = s3Tf[:, p1]
        s2T = np.zeros((256, n2p), np.float32)
        s2T[:, slot2[p2]] = s2Tf[:, p2]
        s1T = np.zeros((128, n3p), np.float32)
        valid = omap >= 0
        s1T[:, valid] = s1Tf[:, omap[valid]]

        g1i = np.full(n1p, 512, np.int64)
        g1i[:k1] = idx1[p1] - C3 * c
        def _z2row(q):
            # physical z2d row of logical stage-1 point q (see z2 store layout)
            gq = q // 1024
            r = q % 1024
            jg = np.minimum(8, (n1p - gq * 1024) // 128)
            return gq * 1024 + (r % 128) * jg + r // 128

        g2i = np.full(n2p, n1p, np.int64)
        g2i[slot2[p2]] = _z2row(pos1[idx2[p2]])

        in_maps.append(
            {
                "featsT": np.ascontiguousarray(featsTf[:, C3 * c : C3 * (c + 1)]),
                "s3T": s3T,
                "s2T": s2T,
                "s1T": s1T,
                "gi1": _wrap_idx(g1i, pl1),
                "gi2": _wrap_idx(g2i, pl2),
                "W1a": W1a,
                "W1b": W1b,
                "W2a": W2a,
                "W2b": W2b,
                "W3a": W3a,
                "W3b": W3b,
                "bn1": bn1,
                "bn2": bn2,
                "b3": b3p,
            }
        )

    return key, in_maps, outmaps


def _install_ntff_hook():
    """The image's antenv lacks axon_hooks; shim it so trace=True can capture
    NTFF profiles through the axon .so (same path trn_boot would register)."""
    import sys as _sys
    import types

    if "antenv.axon_hooks" in _sys.modules:
        return
    mod = types.ModuleType("antenv.axon_hooks")
    holder = {}
    mod.set_axon_ntff_profile_hook = lambda h: holder.__setitem__("h", h)
    mod.get_axon_ntff_profile_hook = lambda: holder.get("h")
    _sys.modules["antenv.axon_hooks"] = mod
    try:
        from trn_agent_boot.trn_boot import _ntff_profile_via_ctypes

        h = _ntff_profile_via_ctypes("/opt/axon/libaxon_pjrt.so")
        if h is not None:
            holder["h"] = h
    except Exception:
        pass


def kernel(_want_trace=False, **inputs):
    import os

    if _want_trace:
        _install_ntff_hook()
    key, in_maps, outmaps = prepare(**inputs)
    stop_after = os.environ.get("K_STOP_AFTER") or None
    key2 = key + (stop_after,)
    nc = _get_program(key2, *key, stop_after)

    res = bass_utils.run_bass_kernel_spmd(
        nc, in_maps, core_ids=list(range(NCORES)), trace=_want_trace
    )

    out = np.empty((N0, 34), np.float32)
    for c in range(NCORES):
        omap = outmaps[c]
        valid = omap >= 0
        out[omap[valid]] = res.results[c]["out"].T[valid]

    if _want_trace:
        kernel._last_trace = res
    return out



# revision 12
# speedup vs baseline: 1.3782x; 1.3782x over previous
"""Trainium2 Bass kernel for nn_Decoder (3-stage point-cloud decoder:
gather-upsample + concat-skip + 1x1conv (+BN+LeakyReLU) x2, final plain conv).

v2 strategy (8-core SPMD), bf16 datapath (rel-err ~4e-3 << 2e-2 gate):
  - Stage-1 gather eliminated: the host gathers feats[idx1] into the per-core
    channel-major input X1T = [feats.T[:, idx1[pts]]; skip3.T[:, pts]]
    ([770, n1p] bf16), so stage 1 is a plain K=770 matmul.
  - Stages run channel-major end to end (no PE transposes): y1T/y2T rows are
    channels, BN stats are free-axis reduces, BN+LReLU is one ACT op with
    per-partition scale/bias.
  - Stage-2 upsample: z2 table [n1p, 128ch] bf16 (256B rows) in DRAM,
    dma_gather(transpose=True) returns the gathered table channel-major.
  - Stage-3 upsample: stage-2 points are host-grouped by stage-3 fanout class,
    so the z3 term is a stride-0 broadcast in the DVE add (baseline trick).
  - Points sharded by the core owning their stage-1 gather source; only the
    two tiny BN-stat AllReduces cross cores.
  - DMA queue discipline: sync queue = pure input streaming (weights, X1T,
    s2T, s1T chunks) so skips stream from t=0; scalar queue = device stores
    (z2d, out); gpsimd = collectives + gathers.
"""

import sys

sys.path.insert(0, "/opt/trn_rl_repo")

import numpy as np
import ml_dtypes

from concourse import bacc, bass, bass_utils, mybir, tile

dt = mybir.dt
AF = mybir.ActivationFunctionType
OP = mybir.AluOpType
AX = mybir.AxisListType

NCORES = 8
EPS = 1e-5
SLOPE = 0.01

N3, N2, N1, N0 = 4096, 16384, 65536, 262144
C3 = 512  # coarse bucket size = N3 // NCORES

GCH = 512  # indices per dma_gather (1024 exceeds the SWDGE packet desc limit)
S1CMAX = 10240  # stage-3 skip chunk columns resident in SBUF

BF = ml_dtypes.bfloat16


def _ceil_to(x, m):
    return ((x + m - 1) // m) * m


def _cls_layout(M):
    """M: tuple of per-fanout-class source-slot counts (index = fanout k).
    Returns (src_off[k], out_off[k], n2p, n3p, s1chunks); s1chunks is a list
    of (out0, olen, [(k, out_piece0, src0, ck), ...]) groups limited to
    S1CMAX output columns, pieces of <=512 cols with k | ck."""
    K = len(M)
    src_off = [0] * K
    out_off = [0] * K
    o = 0
    for k in range(K):
        src_off[k] = o
        o += M[k]
    n2p = o
    o = 0
    for k in range(1, K):
        out_off[k] = o
        o += M[k] * k
    n3p = o

    chunks = []
    cur0, curlen, curpieces = 0, 0, []
    for k in range(1, K):
        blk = M[k] * k
        if blk == 0:
            continue
        assert blk <= S1CMAX, f"class {k} block {blk} exceeds S1CMAX"
        if curlen + blk > S1CMAX and curlen > 0:
            chunks.append((cur0, curlen, curpieces))
            cur0, curlen, curpieces = out_off[k], 0, []
        ck_max = (512 // k) * k
        done = 0
        while done < blk:
            ck = min(ck_max, blk - done)
            curpieces.append((k, out_off[k] + done, src_off[k] + done // k, ck))
            done += ck
        curlen += blk
    if curlen > 0:
        chunks.append((cur0, curlen, curpieces))
    return src_off, out_off, n2p, n3p, chunks


def _wrap_idx(idx, ch):
    """[n] int -> [128, n//16] int16, wrapped per ch-sized block, replicated
    across 16-partition groups (dma_gather idx layout)."""
    n = len(idx)
    out = np.empty((128, n // 16), np.int16)
    for off in range(0, n, ch):
        ln = min(ch, n - off)
        w = idx[off : off + ln].reshape(ln // 16, 16).T.astype(np.int16)
        out[:, off // 16 : (off + ln) // 16] = np.tile(w, (8, 1))
    return out


def _bucket(owner, ncores):
    order = np.argsort(owner, kind="stable")
    counts = np.bincount(owner, minlength=ncores)
    splits = np.split(order, np.cumsum(counts)[:-1])
    pos = np.empty(len(owner), np.int64)
    for c in range(ncores):
        pos[splits[c]] = np.arange(counts[c])
    return splits, pos, counts


def _chunks(n, ch):
    out = []
    off = 0
    while off < n:
        ln = min(ch, n - off)
        out.append((off, ln))
        off += ln
    return out


# ---------------------------------------------------------------------------
# device program
# ---------------------------------------------------------------------------

PHASES = ["y1", "ar1", "z2", "g2", "ar2", "z3", "st3"]


def _build_program(n1p, Mtuple, stop_after=None):
    lim = PHASES.index(stop_after) if stop_after else len(PHASES) - 1

    def on(ph):
        return PHASES.index(ph) <= lim

    src_off, out_off, n2p, n3p, s1chunks = _cls_layout(Mtuple)
    NG = n2p // GCH

    nc = bacc.Bacc(
        "TRN2",
        target_bir_lowering=False,
        debug=False,
        num_devices=NCORES,
        num_swdge_queues=4,
    )

    f32 = dt.float32
    bf16 = dt.bfloat16
    i16 = dt.int16

    # ---- I/O ----
    X1T_h = nc.dram_tensor("X1T", [770, n1p], bf16, kind="ExternalInput")
    s2T_h = nc.dram_tensor("s2T", [256, n2p], bf16, kind="ExternalInput")
    s1T_h = nc.dram_tensor("s1T", [128, n3p], bf16, kind="ExternalInput")
    gi2_h = nc.dram_tensor("gi2", [128, n2p // 16], i16, kind="ExternalInput")
    W1m_h = nc.dram_tensor("W1m", [768, 128], bf16, kind="ExternalInput")
    W1x_h = nc.dram_tensor("W1x", [2, 128], bf16, kind="ExternalInput")
    W1s_h = nc.dram_tensor("W1s", [768, 1], bf16, kind="ExternalInput")
    W1sx_h = nc.dram_tensor("W1sx", [2, 1], bf16, kind="ExternalInput")
    W2am_h = nc.dram_tensor("W2am", [128, 64], bf16, kind="ExternalInput")
    W2ax_h = nc.dram_tensor("W2ax", [1, 64], bf16, kind="ExternalInput")
    W2b_h = nc.dram_tensor("W2b", [256, 64], bf16, kind="ExternalInput")
    W3a_h = nc.dram_tensor("W3a", [64, 64], bf16, kind="ExternalInput")
    W3b_h = nc.dram_tensor("W3b", [128, 34], bf16, kind="ExternalInput")
    bn1_h = nc.dram_tensor("bn1", [128, 2], f32, kind="ExternalInput")
    bn1x_h = nc.dram_tensor("bn1x", [1, 2], f32, kind="ExternalInput")
    bn2_h = nc.dram_tensor("bn2", [64, 2], f32, kind="ExternalInput")
    b3_h = nc.dram_tensor("b3", [64, 1], f32, kind="ExternalInput")
    out_h = nc.dram_tensor("out", [34, n3p], bf16, kind="ExternalOutput")

    c1s = _chunks(n1p, 512)  # stage-1 column chunks
    c2s = _chunks(n2p, 512)  # stage-2 column chunks

    def bn_scalars(sb, nc, stats, gbe, n_true, P, name):
        """stats [P,2]=(sum,sumsq) -> s,t [P,1]: s=g*rsqrt(var+eps),
        t=be-mean*s."""
        mean = sb.tile([P, 1], f32, tag=f"{name}_mean")
        ms = sb.tile([P, 1], f32, tag=f"{name}_ms")
        nc.vector.tensor_scalar(mean[:], stats[:, 0:1], 1.0 / n_true, None, OP.mult)
        nc.vector.tensor_scalar(ms[:], stats[:, 1:2], 1.0 / n_true, None, OP.mult)
        var = sb.tile([P, 1], f32, tag=f"{name}_var")
        nc.vector.tensor_tensor(var[:], mean[:], mean[:], OP.mult)
        nc.vector.tensor_tensor(var[:], ms[:], var[:], OP.subtract)
        nc.vector.tensor_scalar(var[:], var[:], EPS, None, OP.add)
        std = sb.tile([P, 1], f32, tag=f"{name}_std")
        nc.scalar.activation(std[:], var[:], AF.Sqrt)
        s = sb.tile([P, 1], f32, tag=f"{name}_s")
        nc.vector.reciprocal(s[:], std[:])
        nc.vector.tensor_tensor(s[:], s[:], gbe[:, 0:1], OP.mult)
        t = sb.tile([P, 1], f32, tag=f"{name}_t")
        nc.vector.tensor_tensor(t[:], mean[:], s[:], OP.mult)
        nc.vector.tensor_tensor(t[:], gbe[:, 1:2], t[:], OP.subtract)
        return s, t

    with tile.TileContext(nc) as tc:
        from contextlib import ExitStack

        octx = ExitStack()
        with octx:
            sb = octx.enter_context(tc.tile_pool(name="persist", bufs=1))
            dram = octx.enter_context(tc.tile_pool(name="dram", bufs=1, space="DRAM"))

            zrow = sb.tile([1, 128], bf16)
            nc.gpsimd.memset(zrow[:], 0.0)

            # ---- weights / params (sync queue, first) ----
            W1m = sb.tile([128, 6, 128], bf16)
            nc.sync.dma_start(
                W1m[:], W1m_h.ap().rearrange("(g p) c -> p g c", g=6, p=128)
            )
            W1x = sb.tile([2, 128], bf16)
            nc.sync.dma_start(W1x[:], W1x_h.ap())
            W1s = sb.tile([128, 6, 1], bf16)
            nc.sync.dma_start(
                W1s[:], W1s_h.ap().rearrange("(g p) c -> p g c", g=6, p=128)
            )
            W1sx = sb.tile([2, 1], bf16)
            nc.sync.dma_start(W1sx[:], W1sx_h.ap())
            W2am = sb.tile([128, 64], bf16)
            nc.sync.dma_start(W2am[:], W2am_h.ap())
            W2ax = sb.tile([1, 64], bf16)
            nc.sync.dma_start(W2ax[:], W2ax_h.ap())
            W2b = sb.tile([128, 2, 64], bf16)
            nc.sync.dma_start(
                W2b[:], W2b_h.ap().rearrange("(g p) c -> p g c", g=2, p=128)
            )
            W3a = sb.tile([64, 64], bf16)
            nc.sync.dma_start(W3a[:], W3a_h.ap())
            W3b = sb.tile([128, 34], bf16)
            nc.sync.dma_start(W3b[:], W3b_h.ap())
            bn1 = sb.tile([128, 2], f32)
            nc.sync.dma_start(bn1[:], bn1_h.ap())
            bn1x = sb.tile([1, 2], f32)
            nc.sync.dma_start(bn1x[:], bn1x_h.ap())
            bn2 = sb.tile([64, 2], f32)
            nc.sync.dma_start(bn2[:], bn2_h.ap())
            b3p = sb.tile([64, 1], f32)
            nc.sync.dma_start(b3p[:], b3_h.ap())
            gi2 = sb.tile([128, n2p // 16], i16)
            nc.sync.dma_start(gi2[:], gi2_h.ap())

            # cross-stage SBUF (persistent)
            x1T = sb.tile([128, n1p], bf16)
            x1x = sb.tile([1, n1p], bf16)
            nT1 = n1p // 128
            zsb2 = sb.tile([128, nT1, 64], bf16)  # z2 staging
            x2T = sb.tile([64, n2p], bf16)
            z3T = sb.tile([64, n2p], bf16)

            # DRAM z2 table ([n1p+1, 128] bf16; row n1p = zero pad row)
            z2d = dram.tile([n1p + 1, 128], bf16)
            nc.scalar.dma_start(z2d[n1p : n1p + 1, :], zrow[:])

            # ================= stage 1 =================
            x1ctx = ExitStack()
            x1pool = x1ctx.enter_context(tc.tile_pool(name="x1pool", bufs=1))
            X1T = x1pool.tile([128, 6, n1p], bf16)
            X1x = x1pool.tile([2, n1p], bf16)
            y1T = x1pool.tile([128, n1p], f32)
            y1x = x1pool.tile([1, n1p], f32)
            NC1 = len(c1s)
            sm1 = sb.tile([128, NC1], f32)
            ss1 = sb.tile([128, NC1], f32)
            smx = sb.tile([1, NC1], f32)
            ssx = sb.tile([1, NC1], f32)

            # stream X1T by column chunk (all 6 K-groups per chunk)
            for ci, (c0, cl) in enumerate(c1s):
                nc.sync.dma_start(
                    X1T[:, :, c0 : c0 + cl],
                    X1T_h.ap()[0:768, c0 : c0 + cl].rearrange(
                        "(g p) n -> p g n", g=6, p=128
                    ),
                )
            nc.sync.dma_start(X1x[:], X1T_h.ap()[768:770, :])

            if on("y1"):
                with (
                    nc.named_scope("ph_y1"),
                    tc.tile_pool(name="y1ps", bufs=2, space="PSUM") as y1ps,
                    tc.tile_pool(name="y1px", bufs=2, space="PSUM") as y1px,
                    tc.tile_pool(name="sq1", bufs=2) as sq1,
                ):
                    for ci, (c0, cl) in enumerate(c1s):
                        ps = y1ps.tile([128, 512], f32, tag="ps")
                        for g in range(6):
                            nc.tensor.matmul(
                                ps[:, :cl],
                                W1m[:, g, :],
                                X1T[:, g, c0 : c0 + cl],
                                start=(g == 0),
                                stop=False,
                            )
                        nc.tensor.matmul(
                            ps[:, :cl], W1x[:], X1x[:, c0 : c0 + cl],
                            start=False, stop=True,
                        )
                        psx = y1px.tile([1, 512], f32, tag="psx")
                        for g in range(6):
                            nc.tensor.matmul(
                                psx[:, :cl],
                                W1s[:, g, :],
                                X1T[:, g, c0 : c0 + cl],
                                start=(g == 0),
                                stop=False,
                            )
                        nc.tensor.matmul(
                            psx[:, :cl], W1sx[:], X1x[:, c0 : c0 + cl],
                            start=False, stop=True,
                        )
                        # evacuate + fused sum
                        nc.vector.tensor_scalar(
                            y1T[:, c0 : c0 + cl], ps[:, :cl], 1.0, None,
                            OP.mult, OP.add, accum_out=sm1[:, ci : ci + 1],
                        )
                        nc.vector.tensor_scalar(
                            y1x[:, c0 : c0 + cl], psx[:, :cl], 1.0, None,
                            OP.mult, OP.add, accum_out=smx[:, ci : ci + 1],
                        )
                        scr = sq1.tile([128, 512], f32, tag="scr")
                        nc.scalar.activation(
                            scr[:, :cl], y1T[:, c0 : c0 + cl], AF.Square,
                            accum_out=ss1[:, ci : ci + 1],
                        )
                        scrx = sq1.tile([1, 512], f32, tag="scrx")
                        nc.scalar.activation(
                            scrx[:, :cl], y1x[:, c0 : c0 + cl], AF.Square,
                            accum_out=ssx[:, ci : ci + 1],
                        )

            if on("ar1"):
                st1m = sb.tile([128, 2], f32)
                st1x = sb.tile([1, 2], f32)
                nc.vector.tensor_reduce(st1m[:, 0:1], sm1[:], AX.X, OP.add)
                nc.vector.tensor_reduce(st1m[:, 1:2], ss1[:], AX.X, OP.add)
                nc.vector.tensor_reduce(st1x[:, 0:1], smx[:], AX.X, OP.add)
                nc.vector.tensor_reduce(st1x[:, 1:2], ssx[:], AX.X, OP.add)
                ar1i = dram.tile([129, 2], f32, tag="ar1i")
                ar1o = dram.tile([129, 2], f32, tag="ar1o")
                nc.gpsimd.dma_start(ar1i[0:128, :], st1m[:])
                nc.gpsimd.dma_start(ar1i[128:129, :], st1x[:])
                nc.gpsimd.collective_compute(
                    "AllReduce", OP.add,
                    ins=[ar1i.opt()], outs=[ar1o.opt()],
                    replica_groups=[list(range(NCORES))],
                )
                rst1m = sb.tile([128, 2], f32)
                rst1x = sb.tile([1, 2], f32)
                nc.gpsimd.dma_start(rst1m[:], ar1o[0:128, :])
                nc.gpsimd.dma_start(rst1x[:], ar1o[128:129, :])
                s1m, t1m = bn_scalars(sb, nc, rst1m, bn1, float(N2), 128, "bn1m")
                s1x, t1x = bn_scalars(sb, nc, rst1x, bn1x, float(N2), 1, "bn1x")
                nc.scalar.activation(
                    x1T[:], y1T[:], AF.Lrelu, bias=t1m[:], scale=s1m[:], alpha=SLOPE
                )
                nc.scalar.activation(
                    x1x[:], y1x[:], AF.Lrelu, bias=t1x[:], scale=s1x[:], alpha=SLOPE
                )
            x1ctx.close()

            # ================= z2 table =================
            if on("z2"):
                with (
                    nc.named_scope("ph_z2"),
                    tc.tile_pool(name="z2ps", bufs=2, space="PSUM") as z2ps,
                ):
                    for t in range(nT1):
                        ps = z2ps.tile([128, 64], f32, tag="ps")
                        nc.tensor.matmul(
                            ps[:], x1T[:, t * 128 : (t + 1) * 128], W2am[:],
                            start=True, stop=False,
                        )
                        nc.tensor.matmul(
                            ps[:], x1x[:, t * 128 : (t + 1) * 128], W2ax[:],
                            start=False, stop=True,
                        )
                        nc.scalar.activation(zsb2[:, t, :], ps[:], AF.Copy)
                    nc.scalar.dma_start(
                        z2d[0:n1p, 0:64].rearrange("(t p) c -> p t c", p=128, t=nT1),
                        zsb2[:, 0:nT1, :],
                    )

            # ================= gather + y2 + stats2 =================
            y2ctx = ExitStack()
            y2pool = y2ctx.enter_context(tc.tile_pool(name="y2pool", bufs=1))
            y2T = y2pool.tile([64, n2p], bf16)
            if on("g2"):
                with (
                    nc.named_scope("ph_g2"),
                    tc.tile_pool(name="y2ps", bufs=4, space="PSUM") as y2ps,
                    tc.tile_pool(name="zg2", bufs=4) as zg2p,
                    tc.tile_pool(name="s2c", bufs=3) as s2cp,
                    tc.tile_pool(name="sq2", bufs=2) as sq2,
                ):
                    NC2 = len(c2s)
                    sm2 = sb.tile([64, NC2], f32)
                    ss2 = sb.tile([64, NC2], f32)
                    S2CH = 2048
                    for g in range(NG):
                        if (g * GCH) % S2CH == 0:
                            p0 = g * GCH
                            s2c = s2cp.tile([128, 2, S2CH], bf16, tag="s2c")
                            nc.sync.dma_start(
                                s2c[:],
                                s2T_h.ap()[:, p0 : p0 + S2CH].rearrange(
                                    "(g p) n -> p g n", g=2, p=128
                                ),
                            )
                        zg2 = zg2p.tile([128, 1, GCH], bf16, tag="zg2")
                        nc.gpsimd.dma_gather(
                            zg2[:],
                            z2d[:],
                            gi2[:, g * (GCH // 16) : (g + 1) * (GCH // 16)],
                            GCH,
                            GCH,
                            128,
                            transpose=True,
                            queue_num=0,
                        )
                        for h in range(GCH // 512):
                            ci = (g * GCH + h * 512) // 512
                            c0 = ci * 512
                            l0 = c0 - p0
                            ps = y2ps.tile([64, 512], f32, tag="ps")
                            for kk in range(2):
                                nc.tensor.matmul(
                                    ps[:],
                                    W2b[:, kk, :],
                                    s2c[:, kk, l0 : l0 + 512],
                                    start=(kk == 0),
                                    stop=(kk == 1),
                                )
                            nc.vector.tensor_tensor(
                                y2T[:, c0 : c0 + 512],
                                ps[:],
                                zg2[0:64, 0, h * 512 : (h + 1) * 512],
                                OP.add,
                            )
                            nc.vector.tensor_reduce(
                                sm2[:, ci : ci + 1], y2T[:, c0 : c0 + 512],
                                AX.X, OP.add,
                            )
                            scr = sq2.tile([64, 512], f32, tag="scr")
                            nc.scalar.activation(
                                scr[:], y2T[:, c0 : c0 + 512], AF.Square,
                                accum_out=ss2[:, ci : ci + 1],
                            )

            if on("ar2"):
                st2m = sb.tile([64, 2], f32)
                nc.vector.tensor_reduce(st2m[:, 0:1], sm2[:], AX.X, OP.add)
                nc.vector.tensor_reduce(st2m[:, 1:2], ss2[:], AX.X, OP.add)
                ar2i = dram.tile([64, 2], f32, tag="ar2i")
                ar2o = dram.tile([64, 2], f32, tag="ar2o")
                nc.gpsimd.dma_start(ar2i[:], st2m[:])
                nc.gpsimd.collective_compute(
                    "AllReduce", OP.add,
                    ins=[ar2i.opt()], outs=[ar2o.opt()],
                    replica_groups=[list(range(NCORES))],
                )
                rst2 = sb.tile([64, 2], f32)
                nc.gpsimd.dma_start(rst2[:], ar2o[:])
                s2s, t2s = bn_scalars(sb, nc, rst2, bn2, float(N1), 64, "bn2")

            # ================= x2 + z3 =================
            if on("z3"):
                with (
                    nc.named_scope("ph_z3"),
                    tc.tile_pool(name="z3ps", bufs=2, space="PSUM") as z3ps,
                ):
                    for c0, cl in c2s:
                        nc.scalar.activation(
                            x2T[:, c0 : c0 + cl], y2T[:, c0 : c0 + cl], AF.Lrelu,
                            bias=t2s[:], scale=s2s[:], alpha=SLOPE,
                        )
                        ps = z3ps.tile([64, 512], f32, tag="ps")
                        nc.tensor.matmul(
                            ps[:, :cl], W3a[:], x2T[:, c0 : c0 + cl],
                            start=True, stop=True,
                        )
                        nc.scalar.activation(
                            z3T[:, c0 : c0 + cl], ps[:, :cl], AF.Identity, bias=b3p[:]
                        )
            y2ctx.close()

            # ================= stage 3 =================
            if on("st3"):
                with (
                    nc.named_scope("ph_st3"),
                    tc.tile_pool(name="s1chunk", bufs=2) as s1chunk,
                    tc.tile_pool(name="outp", bufs=2) as outp,
                    tc.tile_pool(name="yps3", bufs=4, space="PSUM") as yps3,
                ):
                    for cols0, clen, pieces in s1chunks:
                        s1c = s1chunk.tile([128, S1CMAX], bf16, tag="s1c")
                        nc.sync.dma_start(
                            s1c[:, :clen], s1T_h.ap()[:, cols0 : cols0 + clen]
                        )
                        ot = outp.tile([34, S1CMAX], bf16, tag="ot")
                        for k, g0, s0, ck in pieces:
                            nj = ck // k
                            l0 = g0 - cols0
                            ps = yps3.tile([34, 512], f32, tag="ps")
                            nc.tensor.matmul(
                                ps[:, :ck], W3b[:], s1c[:, l0 : l0 + ck],
                                start=True, stop=True,
                            )
                            nc.vector.tensor_tensor(
                                ot[:, l0 : l0 + ck].rearrange(
                                    "p (j i) -> p j i", i=k
                                ),
                                ps[:, :ck].rearrange("p (j i) -> p j i", i=k),
                                z3T[0:34, s0 : s0 + nj]
                                .unsqueeze(2)
                                .broadcast_to([34, nj, k]),
                                OP.add,
                            )
                        nc.scalar.dma_start(
                            out_h.ap()[:, cols0 : cols0 + clen], ot[:, :clen]
                        )

    nc.compile()
    return nc


# ---------------------------------------------------------------------------
# host wrapper
# ---------------------------------------------------------------------------

_CACHE = {}


def _get_program(key, *args):
    if key not in _CACHE:
        _CACHE[key] = _build_program(*args)
    return _CACHE[key]


def prepare(
    feats, skip1, skip2, skip3, idx1, idx2, idx3,
    W1, b1, g1, be1, W2, b2, g2, be2, W3, b3,
):
    """Host-side sharding/layout. Returns (build_key, in_maps, outmaps)."""
    feats = np.asarray(feats, np.float32)
    skip1 = np.asarray(skip1, np.float32)
    skip2 = np.asarray(skip2, np.float32)
    skip3 = np.asarray(skip3, np.float32)
    idx1 = np.asarray(idx1, np.int64)
    idx2 = np.asarray(idx2, np.int64)
    idx3 = np.asarray(idx3, np.int64)
    W1 = np.asarray(W1, np.float32)
    W2 = np.asarray(W2, np.float32)
    W3 = np.asarray(W3, np.float32)
    b3 = np.asarray(b3, np.float32)
    g1 = np.asarray(g1, np.float32)
    be1 = np.asarray(be1, np.float32)
    g2 = np.asarray(g2, np.float32)
    be2 = np.asarray(be2, np.float32)

    # ---- locality sharding ----
    own1 = idx1 // C3
    P1, pos1, cnt1 = _bucket(own1, NCORES)
    own2 = own1[idx2]
    P2raw, _, cnt2 = _bucket(own2, NCORES)

    n1p = _ceil_to(cnt1.max(), 128)

    # ---- stage-3 fanout classes over stage-2 points ----
    fan = np.bincount(idx3, minlength=N1)
    KMAX = int(fan.max())
    cnt_ck = np.zeros((NCORES, KMAX + 1), np.int64)
    for c in range(NCORES):
        cnt_ck[c] = np.bincount(fan[P2raw[c]], minlength=KMAX + 1)
    M = cnt_ck.max(axis=0)
    M[0] += _ceil_to(int(M.sum()), 2048) - int(M.sum())
    Mtuple = tuple(int(x) for x in M)

    src_off, out_off, n2p, n3p, _ = _cls_layout(Mtuple)

    # stage-2 slot assignment: class-grouped per core
    slot2 = np.full(N1, -1, np.int64)
    P2 = []
    for c in range(NCORES):
        pts = P2raw[c]
        order = np.argsort(fan[pts], kind="stable")
        pts_sorted = pts[order]
        ks = fan[pts_sorted]
        slots = np.empty(len(pts), np.int64)
        for k in range(KMAX + 1):
            m = ks == k
            slots[m] = src_off[k] + np.arange(int(m.sum()))
        slot2[pts_sorted] = slots
        P2.append(pts_sorted)

    # stage-3 output mapping (CSR over idx3 by source)
    order3 = np.argsort(idx3, kind="stable")
    start = np.zeros(N1 + 1, np.int64)
    np.cumsum(fan, out=start[1:])

    outmaps = []
    for c in range(NCORES):
        omap = np.full(n3p, -1, np.int64)
        pts = P2[c]
        ks = fan[pts]
        for k in range(1, KMAX + 1):
            srcs = pts[ks == k]
            nk = len(srcs)
            if nk == 0:
                continue
            gidx = (start[srcs][:, None] + np.arange(k)[None, :]).reshape(-1)
            omap[out_off[k] : out_off[k] + nk * k] = order3[gidx]
        outmaps.append(omap)

    key = (n1p, Mtuple)

    # shared weights (bf16)
    W1m = np.ascontiguousarray(W1[0:768, 0:128]).astype(BF)
    W1x = np.ascontiguousarray(W1[768:770, 0:128]).astype(BF)
    W1s = np.ascontiguousarray(W1[0:768, 128:129]).astype(BF)
    W1sx = np.ascontiguousarray(W1[768:770, 128:129]).astype(BF)
    W2am = np.ascontiguousarray(W2[0:128]).astype(BF)
    W2ax = np.ascontiguousarray(W2[128:129]).astype(BF)
    W2b = np.ascontiguousarray(W2[129:385]).astype(BF)
    W3a = np.zeros((64, 64), np.float32)
    W3a[:, :34] = W3[:64]
    W3a = W3a.astype(BF)
    W3b = np.ascontiguousarray(W3[64:192]).astype(BF)
    bn1 = np.stack([g1[:128], be1[:128]], 1)
    bn1x = np.stack([g1[128:129], be1[128:129]], 1)
    bn2 = np.stack([g2, be2], 1)
    b3p = np.zeros((64, 1), np.float32)
    b3p[:34, 0] = b3

    featsT = feats.T  # [258, 4096]
    s3Tf = skip3.T  # [512, N2]
    s2Tf = skip2.T.astype(BF)  # [256, N1]
    s1Tf = skip1.T.astype(BF)  # [128, N0]

    in_maps = []
    for c in range(NCORES):
        p1, p2 = P1[c], P2[c]
        k1 = len(p1)
        omap = outmaps[c]

        X1T = np.zeros((770, n1p), BF)
        X1T[0:258, :k1] = featsT[:, idx1[p1]].astype(BF)
        X1T[258:770, :k1] = s3Tf[:, p1].astype(BF)

        s2T = np.zeros((256, n2p), BF)
        s2T[:, slot2[p2]] = s2Tf[:, p2]
        s1T = np.zeros((128, n3p), BF)
        valid = omap >= 0
        s1T[:, valid] = s1Tf[:, omap[valid]]

        g2i = np.full(n2p, n1p, np.int64)
        g2i[slot2[p2]] = pos1[idx2[p2]]

        in_maps.append(
            {
                "X1T": X1T,
                "s2T": s2T,
                "s1T": s1T,
                "gi2": _wrap_idx(g2i, GCH),
                "W1m": W1m, "W1x": W1x, "W1s": W1s, "W1sx": W1sx,
                "W2am": W2am, "W2ax": W2ax, "W2b": W2b,
                "W3a": W3a, "W3b": W3b,
                "bn1": bn1, "bn1x": bn1x, "bn2": bn2, "b3": b3p,
            }
        )

    return key, in_maps, outmaps


def _install_ntff_hook():
    """The image's antenv lacks axon_hooks; shim it so trace=True can capture
    NTFF profiles through the axon .so (same path trn_boot would register)."""
    import sys as _sys
    import types

    if "antenv.axon_hooks" in _sys.modules:
        return
    mod = types.ModuleType("antenv.axon_hooks")
    holder = {}
    mod.set_axon_ntff_profile_hook = lambda h: holder.__setitem__("h", h)
    mod.get_axon_ntff_profile_hook = lambda: holder.get("h")
    _sys.modules["antenv.axon_hooks"] = mod
    try:
        from trn_agent_boot.trn_boot import _ntff_profile_via_ctypes

        h = _ntff_profile_via_ctypes("/opt/axon/libaxon_pjrt.so")
        if h is not None:
            holder["h"] = h
    except Exception:
        pass


def kernel(_want_trace=False, **inputs):
    import os

    if _want_trace:
        _install_ntff_hook()
    key, in_maps, outmaps = prepare(**inputs)
    stop_after = os.environ.get("K_STOP_AFTER") or None
    key2 = key + (stop_after,)
    nc = _get_program(key2, *key, stop_after)

    res = bass_utils.run_bass_kernel_spmd(
        nc, in_maps, core_ids=list(range(NCORES)), trace=_want_trace
    )

    out = np.empty((N0, 34), np.float32)
    for c in range(NCORES):
        omap = outmaps[c]
        valid = omap >= 0
        o = np.asarray(res.results[c]["out"])
        if o.dtype != np.float32:
            o = o.astype(np.float32)
        out[omap[valid]] = o.T[valid]

    if _want_trace:
        kernel._last_trace = res
    return out
